# revision 1
# baseline (speedup 1.0000x reference)
"""Two-layer GCN (AggregationNetwork) on 8 Trainium2 NeuronCores.

Strategy (graph/data parallel, destination-node sharded):
  Host: add self-loops, sort edges by destination, shard destinations across
  8 cores (12544 nodes each, padded). Group each core's edges into 64-dest
  blocks; within each superblock of 8 blocks, split edges into 4 gather
  groups (source-row pair/parity, for int16 dma_gather addressing) and pad
  every (block, group) cell to multiples of 128 edges — uniformly across
  cores so one SPMD program serves all 8.

  Device per core:
    deg   = reduce of host-staged per-node padded weights      (replicated)
    dinv  = 1/sqrt(max(deg, 1))
    u     = dinv * x  (fp16 rows, compact)
    agg1  = segment-sum of dma_gather'ed u rows via one-hot matmuls
    v2    = dinv*relu(dinv*(agg1@W1) + b1)                     (local shard)
    v2    AllGather (fp16) -> v2_full
    agg2  = same segment-sum over gathered v2 rows
    out   = dinv*(agg2@W2) + b2                                (local shard)
"""

import sys
import time

sys.path.insert(0, "/opt/trn_rl_repo")

import numpy as np

import concourse.bass as bass
import concourse.bacc as bacc
import concourse.mybir as mybir
import concourse.tile as tile
from concourse import ap_utils
from concourse.bass import MemorySpace, round_up_to_multiple, exact_div

f32 = mybir.dt.float32
f16 = mybir.dt.float16
i16 = mybir.dt.int16
i32 = mybir.dt.int32

F = 64          # feature dim
D = 64          # destination-block size
NC = 8          # cores
GSB = 8         # blocks per superblock
NGRP = 4        # gather groups (2 pair-ranges x 2 parities)
HALFCAP = 32768


def _dma_gather_small_elem(gp, out_ap, in_ap, idxs_ap, num_idxs, elem_size,
                           elem_step):
    """bass.dma_gather minus the 256-byte elem_size assert (the ucode's
    non-transpose path supports any payload; only the row STRIDE must be a
    multiple of 256B, which elem_step enforces)."""
    assert idxs_ap.dtype == mybir.dt.int16
    assert in_ap.dtype == out_ap.dtype
    assert in_ap.space == MemorySpace.DRAM
    assert idxs_ap.space == MemorySpace.SBUF
    assert out_ap.space == MemorySpace.SBUF
    assert ap_utils.ap_is_contiguous(out_ap.ap[1:])
    assert ap_utils.ap_is_contiguous(idxs_ap.ap[1:])
    assert in_ap.ap[-1][1] == out_ap.ap[-1][1] == elem_size
    assert in_ap.ap[0][0] == elem_step
    stride_bytes = elem_step * mybir.dt.size(in_ap.dtype)
    stride_bytes_256 = exact_div(stride_bytes, 256)
    assert stride_bytes_256 < 256
    _in_ap = gp.lower_ap_dma(in_ap, for_custom_bir_dma=True)
    inst = gp.add_instruction(
        mybir.InstDMAGatherAnt(
            name=gp.bass.get_next_instruction_name(),
            ins=[*_in_ap, gp.lower_ap(idxs_ap),
                 gp.lower_val_access(gp.to_reg(num_idxs))],
            outs=[gp.lower_ap(out_ap)],
            transpose=False,
            num_idxs=num_idxs,
            elem_size=elem_size,
            stride_bytes_256=stride_bytes_256,
            gen_mode=0,
            single_packet=True,
            queue_num=0,
            sbuf_tokens_per_rank=0,
            sbuf_free_dim_per_rank=0,
            sbuf_free_dim_pad_per_rank=0,
            sbuf_byte_offset=0,
        ))
    return inst


# ----------------------------------------------------------------------------
# host-side preprocessing (index routing / data staging only, no FP math)
# ----------------------------------------------------------------------------

def _preprocess(N, edge_index, edge_weight):
    shard = ((N + NC - 1) // NC + 127) // 128 * 128
    npad = NC * shard
    gn = npad // 128
    gs = shard // 128
    nblk = shard // D
    half = npad // 2
    r0cap = min(HALFCAP, half)          # pair rows in range 0

    rows = np.concatenate([edge_index[0].astype(np.int64),
                           np.arange(N, dtype=np.int64)])
    cols = np.concatenate([edge_index[1].astype(np.int64),
                           np.arange(N, dtype=np.int64)])
    ws = np.concatenate([edge_weight.astype(np.float32),
                         np.ones(N, np.float32)])
    order = np.argsort(cols, kind="stable")
    rs, cs, wv = rows[order], cols[order], ws[order]

    # ---- degree staging: per-node padded weight lists (fp16) ----
    cnt_node = np.zeros(npad, np.int64)
    np.add.at(cnt_node, cs, 1)
    K = int(cnt_node.max())
    node_start = np.zeros(npad + 1, np.int64)
    node_start[1:] = np.cumsum(cnt_node)
    pos = np.arange(len(cs))
    within_all = pos - node_start[cs]

    def w_padded(sel_lo, sel_hi):
        nn = sel_hi - sel_lo
        wp = np.zeros((nn, K), np.float32)
        m = (cs >= sel_lo) & (cs < sel_hi)
        wp[cs[m] - sel_lo, within_all[m]] = wv[m]
        g = nn // 128
        return np.ascontiguousarray(
            wp.reshape(g, 128, K).transpose(1, 0, 2).reshape(128, g * K)
        ).astype(np.float16)

    w45_g = w_padded(0, npad)                     # [128, gn*K]
    w45_l = [w_padded(c * shard, (c + 1) * shard) for c in range(NC)]

    # ---- edge schedule: (block, group) cells, uniform across cores ----
    def grp_of(r):
        return np.where(r < 2 * r0cap, 0, 2) + (r % 2)

    nsb = (nblk + GSB - 1) // GSB
    bidx = np.searchsorted(cs, np.arange(0, npad + 1, D)).astype(np.int64)

    cell_edges = {}
    cnt = np.zeros((NC, nblk, NGRP), np.int64)
    for c in range(NC):
        for b in range(nblk):
            s_e, e_e = int(bidx[c * nblk + b]), int(bidx[c * nblk + b + 1])
            g = grp_of(rs[s_e:e_e])
            for q in range(NGRP):
                sel = np.nonzero(g == q)[0] + s_e
                cell_edges[(c, b, q)] = sel
                cnt[c, b, q] = len(sel)

    tbc = np.maximum((cnt.max(axis=0) + 127) // 128, 1)      # [nblk, NGRP]

    # Tile stream is BLOCK-major (for PSUM accumulation locality); gather
    # runs are (superblock, group)-major (for few, large dma_gathers).
    # slot_of_tile maps each stream tile to its slot in its gather run.
    stream = []          # (b, q, is_first_of_block, is_last_of_block)
    for s in range(nsb):
        for b in range(s * GSB, min((s + 1) * GSB, nblk)):
            for q in range(NGRP):
                for t in range(int(tbc[b, q])):
                    first = (q == 0 and t == 0)
                    last = (q == NGRP - 1 and t == tbc[b, NGRP - 1] - 1)
                    stream.append((b, q, first, last))
    t_total = len(stream)

    tile_of_cell = {}
    for j, (b, q, _, _) in enumerate(stream):
        tile_of_cell.setdefault((b, q), []).append(j)

    gather_runs = []     # (run_id -> (grp, [stream tile ids in run order]))
    slot_of_tile = {}    # j -> (run_id, slot)
    for s in range(nsb):
        blks = range(s * GSB, min((s + 1) * GSB, nblk))
        for q in range(NGRP):
            tiles = []
            for b in blks:
                tiles.extend(tile_of_cell[(b, q)])
            rid = len(gather_runs)
            for sl, j in enumerate(tiles):
                slot_of_tile[j] = (rid, sl)
            gather_runs.append((q, tiles))

    per_core = []
    for c in range(NC):
        idx_lin = np.zeros(t_total * 128, np.int64)
        colr_lin = np.zeros(t_total * 128, np.float32)
        w_lin = np.zeros(t_total * 128, np.float32)
        for b in range(nblk):
            for q in range(NGRP):
                sel = cell_edges[(c, b, q)]
                tiles = tile_of_cell[(b, q)]
                n = len(sel)
                dst = np.concatenate(
                    [np.arange(t * 128, t * 128 + 128) for t in tiles])[:n]
                idx_lin[dst] = rs[sel]
                colr_lin[dst] = cs[sel] - (c * shard + b * D)
                w_lin[dst] = wv[sel]
        pair = idx_lin // 2
        rel = np.where(pair < r0cap, pair, pair - r0cap).astype(np.int16)
        rel = rel.reshape(t_total, 128)
        run_order = []
        for q, tiles in gather_runs:
            run_order.extend(tiles)
        idx16 = rel[run_order].reshape(-1)          # run-ordered
        # idxs wrapped in 16 partitions (replicated to 128): [k%16, k//16]
        idx16_w = np.tile(idx16.reshape(t_total * 8, 16).T, (8, 1)).copy()
        per_core.append({
            "idx16": idx16_w,                               # [16, T*8]
            "colrel": colr_lin.reshape(t_total, 128).T.copy(),
            "wedge": w_lin.reshape(t_total, 128).T.copy(),
            "w45l": w45_l[c],
        })

    run_start = np.zeros(len(gather_runs) + 1, np.int64)
    for i, (q, tiles) in enumerate(gather_runs):
        run_start[i + 1] = run_start[i] + len(tiles)
    meta = dict(N=N, shard=shard, npad=npad, gn=gn, gs=gs, nblk=nblk,
                nsb=nsb, K=K, t_total=t_total, r0cap=r0cap,
                stream=stream, gather_runs=gather_runs,
                run_start=[int(v) for v in run_start],
                slot_of_tile=slot_of_tile)
    shared = dict(w45_g=w45_g)
    return meta, shared, per_core, (rs, cs, wv)


# ----------------------------------------------------------------------------
# device program
# ----------------------------------------------------------------------------

def _build_program(meta, dbg=False, stages=4):
    shard, npad, gn, gs = meta["shard"], meta["npad"], meta["gn"], meta["gs"]
    nblk, nsb, K = meta["nblk"], meta["nsb"], meta["K"]
    t_total, r0cap = meta["t_total"], meta["r0cap"]
    stream, gather_runs = meta["stream"], meta["gather_runs"]
    half = npad // 2

    nc = bacc.Bacc("TRN2", target_bir_lowering=False, debug=False,
                   num_devices=NC)

    # ---- I/O ----
    x_d = nc.dram_tensor("x_in", [128, gn * F], f32, kind="ExternalInput").ap()
    w45g_d = nc.dram_tensor("w45_g", [128, gn * K], f16, kind="ExternalInput").ap()
    w45l_d = nc.dram_tensor("w45l", [128, gs * K], f16, kind="ExternalInput").ap()
    idx16_d = nc.dram_tensor("idx16", [128, t_total * 8], i16, kind="ExternalInput").ap()
    colrel_d = nc.dram_tensor("colrel", [128, t_total], f32, kind="ExternalInput").ap()
    wedge_d = nc.dram_tensor("wedge", [128, t_total], f32, kind="ExternalInput").ap()
    iota_d = nc.dram_tensor("iota", [128, F], f32, kind="ExternalInput").ap()
    ident64_d = nc.dram_tensor("ident64", [64, 64], f32, kind="ExternalInput").ap()
    w1e_d = nc.dram_tensor("W1e", [F + 1, F], f32, kind="ExternalInput").ap()
    w2_d = nc.dram_tensor("W2", [F, 1], f32, kind="ExternalInput").ap()
    b2_d = nc.dram_tensor("b2c", [F, 1], f32, kind="ExternalInput").ap()

    out_d = nc.dram_tensor("out", [shard], f32, kind="ExternalOutput").ap()

    # ---- DRAM internals ----
    kind = dict(kind="ExternalOutput") if dbg else {}
    u_dram = nc.dram_tensor("u_dram", [npad, F], f16, **kind).ap()
    v2_bounce = nc.dram_tensor("v2_bounce", [shard, F], f16).ap()
    v2_full = nc.dram_tensor("v2_full", [npad, F], f16,
                             addr_space="Shared").ap()
    dbg_outs = {}
    if dbg:
        dbg_outs["dinv_o"] = nc.dram_tensor(
            "dinv_o", [128, gn], f32, kind="ExternalOutput").ap()
        dbg_outs["dinvloc_o"] = nc.dram_tensor(
            "dinvloc_o", [128, gs], f32, kind="ExternalOutput").ap()
        dbg_outs["v2full_o"] = nc.dram_tensor(
            "v2full_o", [npad, F], f16, kind="ExternalOutput").ap()
        dbg_outs["v2b_o"] = nc.dram_tensor(
            "v2b_o", [shard, F], f16, kind="ExternalOutput").ap()

    # packed pair views of the gather sources: [npad/2, 128] fp16
    u_pack = u_dram.rearrange("(h two) f -> h (two f)", two=2)
    v2_pack = v2_full.rearrange("(h two) f -> h (two f)", two=2)

    def grp_view(pack, q):
        if q < 2 or r0cap >= half:
            r0, r1 = 0, r0cap
        else:
            r0, r1 = r0cap, half
        off = (q % 2) * F
        return pack[r0:r1, off:off + F]

    with tile.TileContext(nc) as tc:
        with tc.tile_pool(name="persist", bufs=1) as pp:
            dinv = pp.tile([128, gn], f32, tag="dinv")
            dinv_loc = pp.tile([128, gs], f32, tag="dinvloc")
            idx16_sb = pp.tile([128, t_total * 8], i16, tag="idx16")
            colrel_sb = pp.tile([128, t_total], f32, tag="colrel")
            wedge_sb = pp.tile([128, t_total], f32, tag="wedge")
            iota_sb = pp.tile([128, F], f32, tag="iota")
            ident64_sb = pp.tile([64, 64], f32, tag="ident")
            w1e_sb = pp.tile([F + 1, F], f32, tag="w1e")
            w2_sb = pp.tile([F, 1], f32, tag="w2")
            b2_sb = pp.tile([F, 1], f32, tag="b2")
            v2_sb = pp.tile([64, nblk * F], f16, tag="v2sb")
            out2_sb = pp.tile([64, nblk], f32, tag="out2")

            for sb_t, dr in ((iota_sb, iota_d), (ident64_sb, ident64_d),
                             (w1e_sb, w1e_d), (w2_sb, w2_d), (b2_sb, b2_d),
                             (idx16_sb, idx16_d), (colrel_sb, colrel_d),
                             (wedge_sb, wedge_d)):
                nc.sync.dma_start(sb_t[:], dr[:])

            # ---------------- stage A: dinv ----------------
            with tc.tile_pool(name="stageA", bufs=1) as sa:
                w45g_sb = sa.tile([128, gn * K], f16, tag="w45g")
                w45l_sb = sa.tile([128, gs * K], f16, tag="w45l")
                deg = sa.tile([128, gn], f32, tag="deg")
                degl = sa.tile([128, gs], f32, tag="degl")
                nc.sync.dma_start(w45g_sb[:], w45g_d[:])
                nc.sync.dma_start(w45l_sb[:], w45l_d[:])
                nc.vector.tensor_reduce(
                    deg[:], w45g_sb[:].rearrange("p (g k) -> p g k", k=K),
                    axis=mybir.AxisListType.X, op=mybir.AluOpType.add)
                nc.vector.tensor_scalar(
                    out=deg[:], in0=deg[:], scalar1=1.0, scalar2=None,
                    op0=mybir.AluOpType.max)
                nc.scalar.activation(deg[:], deg[:],
                                     mybir.ActivationFunctionType.Sqrt)
                nc.vector.reciprocal(dinv[:], deg[:])
                nc.vector.tensor_reduce(
                    degl[:], w45l_sb[:].rearrange("p (g k) -> p g k", k=K),
                    axis=mybir.AxisListType.X, op=mybir.AluOpType.add)
                nc.vector.tensor_scalar(
                    out=degl[:], in0=degl[:], scalar1=1.0, scalar2=None,
                    op0=mybir.AluOpType.max)
                nc.scalar.activation(degl[:], degl[:],
                                     mybir.ActivationFunctionType.Sqrt)
                nc.vector.reciprocal(dinv_loc[:], degl[:])

            if dbg:
                nc.sync.dma_start(dbg_outs["dinv_o"][:], dinv[:])
                nc.sync.dma_start(dbg_outs["dinvloc_o"][:], dinv_loc[:])

            # ---------------- stage B: u = dinv * x (fp16) ----------------
            with tc.tile_pool(name="stageB", bufs=2) as sbp:
                uc = 98
                u_tm = u_dram.rearrange("(g p) f -> p g f", p=128)
                for g0 in range(0, gn, uc):
                    g1 = min(g0 + uc, gn)
                    w = (g1 - g0) * F
                    xt = sbp.tile([128, uc * F], f32, tag="xt")
                    ut = sbp.tile([128, uc * F], f16, tag="ut")
                    nc.sync.dma_start(xt[:, :w], x_d[:, g0 * F:g1 * F])
                    for g in range(g0, g1):
                        sl = slice((g - g0) * F, (g - g0 + 1) * F)
                        nc.vector.tensor_scalar(
                            out=ut[:, sl], in0=xt[:, sl],
                            scalar1=dinv[:, g:g + 1], scalar2=None,
                            op0=mybir.AluOpType.mult)
                    nc.sync.dma_start(
                        u_tm[:, g0:g1, :],
                        ut[:, :w].rearrange("p (g f) -> p g f", f=F))

            # ---------------- aggregation pass ----------------
            tile_of_cell_all = {}
            for j, (b, q, _, _) in enumerate(stream):
                tile_of_cell_all.setdefault(b, []).append(j)
            run_start = meta["run_start"]
            slot_of_tile = meta["slot_of_tile"]
            max_run = max(len(t) for _, t in gather_runs)

            def agg_pass(src_views, post_block, s_is_lhs):
                with (
                    tc.tile_pool(name="gather", bufs=6) as gpl,
                    tc.tile_pool(name="sbuild", bufs=8) as spool,
                    tc.tile_pool(name="aggps", bufs=4, space="PSUM") as aggp,
                    tc.tile_pool(name="postps", bufs=2, space="PSUM") as postp,
                    tc.tile_pool(name="post", bufs=3) as postsb,
                ):
                    SUBRUN = 8   # dma_gather is capped at 1024 indices
                    for s in range(nsb):
                        blks = list(range(s * GSB, min((s + 1) * GSB, nblk)))
                        # issue the superblock's gathers in <=8-tile sub-runs
                        bufs = {}
                        for ri in range(s * NGRP, (s + 1) * NGRP):
                            q, tiles = gather_runs[ri]
                            ntiles = len(tiles)
                            if ntiles == 0:
                                continue
                            r0 = run_start[ri]
                            for sr0 in range(0, ntiles, SUBRUN):
                                sr1 = min(sr0 + SUBRUN, ntiles)
                                nt = sr1 - sr0
                                ut = gpl.tile([128, SUBRUN, F], f16,
                                              tag=f"gath{q}")
                                _dma_gather_small_elem(
                                    nc.gpsimd, ut[:, :nt, :], src_views[q],
                                    idx16_sb[:, (r0 + sr0) * 8:
                                             (r0 + sr1) * 8],
                                    nt * 128, F, 2 * F)
                                bufs[(ri, sr0 // SUBRUN)] = ut
                        # block-major matmul stream
                        for b in blks:
                            agg_ps = aggp.tile(
                                [D, F] if s_is_lhs else [F, D], f32,
                                tag="agg")
                            for j in tile_of_cell_all[b]:
                                bb, q, first, last = stream[j]
                                rid, sl = slot_of_tile[j]
                                ut = bufs[(rid, sl // SUBRUN)]
                                sl = sl % SUBRUN
                                st = spool.tile([128, D], f16, tag="sw")
                                nc.vector.tensor_scalar(
                                    out=st[:], in0=iota_sb[:],
                                    scalar1=colrel_sb[:, j:j + 1],
                                    scalar2=wedge_sb[:, j:j + 1],
                                    op0=mybir.AluOpType.is_equal,
                                    op1=mybir.AluOpType.mult)
                                if s_is_lhs:
                                    nc.tensor.matmul(
                                        agg_ps[:], st[:], ut[:, sl, :],
                                        start=first, stop=last)
                                else:
                                    nc.tensor.matmul(
                                        agg_ps[:], ut[:, sl, :], st[:],
                                        start=first, stop=last)
                            post_block(b, agg_ps[:], postp, postsb)

            # ---------------- pass 1: conv1 + relu + v2 ----------------
            def post1(b, agg_ap, postp, postsb):
                podd = (b % 2) * 64
                g0 = b // 2
                dv = dinv_loc[podd:podd + 64, g0:g0 + 1]
                scaled_sb = postsb.tile([D, F], f32, tag="scaled")
                nc.scalar.activation(scaled_sb[:], agg_ap,
                                     mybir.ActivationFunctionType.Copy,
                                     scale=dv)
                tr_ps = postp.tile([F, D], f32, tag="tr")
                nc.tensor.transpose(tr_ps[:], scaled_sb[:], ident64_sb[:])
                tr_sb = postsb.tile([F + 1, D], f32, tag="trsb")
                nc.scalar.copy(tr_sb[0:F, :], tr_ps[:])
                nc.gpsimd.memset(tr_sb[F:F + 1, :], 1.0)
                h1_ps = postp.tile([D, F], f32, tag="h1")
                nc.tensor.matmul(h1_ps[:], tr_sb[:], w1e_sb[:],
                                 start=True, stop=True)
                nc.vector.tensor_scalar(
                    out=v2_sb[:, b * F:(b + 1) * F], in0=h1_ps[:],
                    scalar1=0.0, scalar2=dv,
                    op0=mybir.AluOpType.max, op1=mybir.AluOpType.mult)

            nc.any.memset(out2_sb[:], 0.0)
            if stages >= 2:
                agg_pass([grp_view(u_pack, q) for q in range(NGRP)],
                         post1, s_is_lhs=True)

            # v2 out + AllGather (natural row order)
            if stages >= 2:
                nc.sync.dma_start(
                    v2_bounce.rearrange("(b d) f -> d b f", d=64),
                    v2_sb[:].rearrange("d (b f) -> d b f", f=F))
            if stages >= 3:
                nc.gpsimd.collective_compute(
                    "AllGather", mybir.AluOpType.bypass,
                    replica_groups=[list(range(NC))],
                    ins=[v2_bounce.opt()], outs=[v2_full.opt()])
            if dbg and stages >= 3:
                nc.gpsimd.dma_start(dbg_outs["v2full_o"][:], v2_full[:])
            if dbg and stages >= 2:
                nc.gpsimd.dma_start(dbg_outs["v2b_o"][:], v2_bounce[:])

            # ---------------- pass 2: conv2 + output ----------------
            def post2(b, agg_ap, postp, postsb):
                podd = (b % 2) * 64
                g0 = b // 2
                agg_sb = postsb.tile([F, D], f32, tag="aggsb2")
                nc.scalar.copy(agg_sb[:], agg_ap)
                o_ps = postp.tile([D, 1], f32, tag="o2")
                nc.tensor.matmul(o_ps[:], agg_sb[:], w2_sb[:],
                                 start=True, stop=True)
                nc.vector.tensor_scalar(
                    out=out2_sb[:, b:b + 1], in0=o_ps[:],
                    scalar1=dinv_loc[podd:podd + 64, g0:g0 + 1],
                    scalar2=b2_sb[0:64, :], op0=mybir.AluOpType.mult,
                    op1=mybir.AluOpType.add)

            if stages >= 4:
                agg_pass([grp_view(v2_pack, q) for q in range(NGRP)],
                         post2, s_is_lhs=False)

            nc.sync.dma_start(out_d.rearrange("(d b) -> d b", d=64),
                              out2_sb[:])

    nc.compile()
    return nc


# ----------------------------------------------------------------------------
# entry / staging
# ----------------------------------------------------------------------------

_CACHE = {}


def _get_program(meta_key, meta):
    if meta_key not in _CACHE:
        _CACHE[meta_key] = _build_program(meta)
    return _CACHE[meta_key]


def _make_in_maps(meta, shared, per_core, x, W1, b1, W2, b2):
    npad = meta["npad"]
    N = meta["N"]

    gn = npad // 128
    x_pad = np.zeros((npad, F), np.float32)
    x_pad[:N] = np.asarray(x, np.float32)
    x_tm = np.ascontiguousarray(
        x_pad.reshape(gn, 128, F).transpose(1, 0, 2).reshape(128, gn * F))

    iota = np.tile(np.arange(F, dtype=np.float32), (128, 1))
    ident64 = np.eye(64, dtype=np.float32)
    w1e_np = np.concatenate([np.asarray(W1, np.float32),
                             np.asarray(b1, np.float32).reshape(1, F)])
    w2_np = np.asarray(W2, np.float32).reshape(F, 1)
    b2_np = np.full((F, 1), np.asarray(b2, np.float32).reshape(-1)[0],
                    np.float32)

    in_maps = []
    for c in range(NC):
        pc = per_core[c]
        in_maps.append({
            "x_in": x_tm,
            "w45_g": shared["w45_g"],
            "w45l": pc["w45l"],
            "idx16": pc["idx16"],
            "colrel": pc["colrel"],
            "wedge": pc["wedge"],
            "iota": iota,
            "ident64": ident64,
            "W1e": w1e_np,
            "W2": w2_np,
            "b2c": b2_np,
        })
    return in_maps


def _unshard(meta, outs):
    shard, npad, nblk, N = meta["shard"], meta["npad"], meta["nblk"], meta["N"]
    out = np.empty((npad,), np.float32)
    for c in range(NC):
        out[c * shard:(c + 1) * shard] = (
            np.asarray(outs[c]).reshape(64, nblk).T.ravel())
    return out[:N].reshape(N, 1)


def _run(N, x, edge_index, edge_weight, W1, b1, W2, b2):
    from concourse.bass_utils import run_bass_kernel_spmd

    meta, shared, per_core, _ = _preprocess(N, edge_index, edge_weight)
    meta_key = (N, edge_index.shape[1])
    nc = _get_program(meta_key, meta)
    in_maps = _make_in_maps(meta, shared, per_core, x, W1, b1, W2, b2)
    res = run_bass_kernel_spmd(nc, in_maps, core_ids=list(range(NC)))
    return _unshard(meta, [res.results[c]["out"] for c in range(NC)])


def kernel(x, edge_index, edge_weight, W1, b1, W2, b2):
    x = np.asarray(x)
    return _run(100000, x, np.asarray(edge_index), np.asarray(edge_weight),
                np.asarray(W1), np.asarray(b1), np.asarray(W2),
                np.asarray(b2))


def bench(inputs, iters=30, N=100000):
    """Wall-clock the SPMD executable with device-resident inputs."""
    import jax
    from jax.sharding import Mesh, PartitionSpec, NamedSharding
    from jax.experimental.shard_map import shard_map
    from concourse import bass2jax
    import concourse.mybir as mb

    meta, shared, per_core, _ = _preprocess(
        N, np.asarray(inputs["edge_index"]), np.asarray(inputs["edge_weight"]))
    meta_key = (N, np.asarray(inputs["edge_index"]).shape[1])
    nc = _get_program(meta_key, meta)
    in_maps = _make_in_maps(meta, shared, per_core, inputs["x"],
                            inputs["W1"], inputs["b1"], inputs["W2"],
                            inputs["b2"])

    bass2jax.install_neuronx_cc_hook()
    in_names, out_names, out_avals, zero_outs = [], [], [], []
    part_name = (nc.partition_id_tensor.name
                 if nc.partition_id_tensor else None)
    for alloc in nc.m.functions[0].allocations:
        if not isinstance(alloc, mb.MemoryLocationSet):
            continue
        name = alloc.memorylocations[0].name
        if alloc.kind == "ExternalInput":
            if name != part_name:
                in_names.append(name)
        elif alloc.kind == "ExternalOutput":
            out_names.append(name)
            shape = tuple(alloc.tensor_shape)
            dtype = mb.dt.np(alloc.dtype)
            out_avals.append(jax.core.ShapedArray(shape, dtype))
            zero_outs.append(np.zeros(shape, dtype))
    n_params = len(in_names)
    all_in_names = in_names + out_names
    if part_name is not None:
        all_in_names = all_in_names + [part_name]

    def _body(*args):
        operands = list(args)
        if part_name is not None:
            operands.append(bass2jax.partition_id_tensor())
        outs = bass2jax._bass_exec_p.bind(
            *operands, out_avals=tuple(out_avals),
            in_names=tuple(all_in_names), out_names=tuple(out_names),
            lowering_input_output_aliases=(),
            sim_require_finite=True, sim_require_nnan=True, nc=nc)
        return tuple(outs)

    devices = jax.devices()[:NC]
    mesh = Mesh(np.asarray(devices), ("core",))
    n_outs = len(out_names)
    sharded = jax.jit(
        shard_map(_body, mesh=mesh,
                  in_specs=(PartitionSpec("core"),) * (n_params + n_outs),
                  out_specs=(PartitionSpec("core"),) * n_outs,
                  check_rep=False),
        keep_unused=True)

    shard_spec = NamedSharding(mesh, PartitionSpec("core"))
    concat_in = [
        jax.device_put(
            np.concatenate([np.asarray(in_maps[c][nm]) for c in range(NC)],
                           axis=0), shard_spec)
        for nm in in_names
    ]
    concat_zero = [
        jax.device_put(np.concatenate([z] * NC, axis=0), shard_spec)
        for z in zero_outs
    ]

    r = sharded(*concat_in, *concat_zero)
    jax.block_until_ready(r)

    times = []
    for _ in range(iters):
        t0 = time.perf_counter()
        r = sharded(*concat_in, *concat_zero)
        jax.block_until_ready(r)
        times.append(time.perf_counter() - t0)
    times.sort()
    return times[0] * 1e9

